# revision 15
# baseline (speedup 1.0000x reference)
"""Trainium2 Bass kernel for CausalSelfAttention with KV-prefix cache (v3).

Problem (hardcoded): B=2, T=2048, C=1024, H=16, D=64, P=2048.
Sharding: 8 cores = 2 (batch) x 4 (head groups of 4 heads).

v3 design vs v2 (354us measured):
 - ScalarE (exp) is the hard floor (~1 elem/cycle/lane @1.2GHz over 27M
   score elements ~= 178us busy).  v2 lost ~34us before the first exp,
   ~54us to ACT gaps at pair boundaries, and ~48us of tail after the
   last exp.  v3 restructures the whole kernel as ONE flat chunk stream
   with a uniform software pipeline: at slot g we emit scores(g), exp(g)
   and AV(g-2).  The 2-slot AV lag crosses segment (tb,pr) boundaries,
   so the PE always has independent work and ACT never waits.
 - causal trim: diagonal chunks only compute/exp/mask the query range
   [j*128, 512) (saves ~12% of exp and score/AV streaming).
 - QKV projection blocks, output-projection blocks and out DMAs are
   woven into the stream with a unit-cost budget + deadlines; only the
   nb=3 projection remains after the stream.
 - tail per segment: one [65,2,512] PSUM->SBUF copy, 2x
   reciprocal_approx_fast (DVE custom op, ~5x faster than reciprocal),
   2x gpsimd partition_broadcast (Pool, idle engine) and 2 DVE
   normalize mults.  No PE broadcast matmuls, no 3.3us reciprocals.
 - ScalarE act-table preload + 16 PE warmup matmuls during the initial
   DMA window (HAM warm, no 2.7us table load on the first real exp).
 - DMA issue spread across SP / ACT / Pool queues so the critical
   first tiles (wqk, xt block0, ckt pair0) land ASAP.
"""

import numpy as np
import ml_dtypes
from collections import deque
from contextlib import ExitStack

import concourse.bacc as bacc
import concourse.tile as tile
import concourse.mybir as mybir
from concourse.bass_utils import run_bass_kernel_spmd

F32 = mybir.dt.float32
F32R = mybir.dt.float32r
BF16 = mybir.dt.bfloat16
EXP = mybir.ActivationFunctionType.Exp
MULT = mybir.AluOpType.mult

B, T, C, H, D, P = 2, 2048, 1024, 16, 64, 2048
HPC = 4            # heads per core
NPAIR = 2          # head pairs per core
TQ = 512           # query block (matmul moving dim)
KC = 128           # key chunk (PSUM partition dim)
SCALE = 1.0 / np.sqrt(D)

NT = T // TQ       # 4  query blocks
NPC = P // KC      # 16 prefix key chunks
NCK = C // 128     # 8  C contraction chunks
NTC = T // 128     # 16 current-key 128-chunks

USE_POOL_BCAST = False  # extended-ISA ops are not supported by this runtime
                        # (verified: CoreSim passes, HW returns garbage)


def build_kernel(n_cores=8):
    nc = bacc.Bacc("TRN2", target_bir_lowering=False, debug=False,
                   num_devices=n_cores)

    xt = nc.dram_tensor("xt", [128, NCK, T], BF16, kind="ExternalInput").ap()
    wqk = nc.dram_tensor("wqk", [128, NCK, 4 * 128], BF16, kind="ExternalInput").ap()
    wv = nc.dram_tensor("wv", [128, NCK, HPC * D], BF16, kind="ExternalInput").ap()
    ckt = nc.dram_tensor("ckt", [NPAIR, 128, P], BF16, kind="ExternalInput").ap()
    cv = nc.dram_tensor("cv", [NPAIR, 128, 2, NPC, 66], BF16, kind="ExternalInput").ap()
    wp = nc.dram_tensor("wp", [NPAIR, 128, C], BF16, kind="ExternalInput").ap()
    masks = nc.dram_tensor("masks", [128, 128], BF16, kind="ExternalInput").ap()
    bsel = nc.dram_tensor("bsel", [65, 64], F32, kind="ExternalInput").ap()
    zrd = nc.dram_tensor("zrd", [65, 2, TQ], F32, kind="ExternalInput").ap()
    out_t = nc.dram_tensor("out_t", [C, T], BF16, kind="ExternalOutput").ap()

    with tile.TileContext(nc) as tc, ExitStack() as top:
        const = top.enter_context(tc.tile_pool(name="const", bufs=1))
        persist = top.enter_context(tc.tile_pool(name="persist", bufs=1))

        # ---- persistent SBUF ---------------------------------------------
        qT = [persist.tile([128, T], BF16, tag=f"qT{i}", name=f"qT{i}") for i in range(NPAIR)]
        kT = [persist.tile([128, T], BF16, tag=f"kT{i}", name=f"kT{i}") for i in range(NPAIR)]
        cktT = [persist.tile([128, P], BF16, tag=f"cktT{i}", name=f"cktT{i}") for i in range(NPAIR)]
        cvt = [persist.tile([128, 2, NPC, 66], BF16, tag=f"cvt{i}", name=f"cvt{i}") for i in range(NPAIR)]
        vt = persist.tile([128, NTC, HPC, 66], BF16, tag="vt", name="vt")
        wpt = [persist.tile([128, C], BF16, tag=f"wpt{i}", name=f"wpt{i}") for i in range(NPAIR)]
        ysb = [persist.tile([128, NT, TQ], BF16, tag=f"ysb{i}", name=f"ysb{i}") for i in range(NPAIR)]
        rd = persist.tile([65, 2, TQ], F32, tag="rd", name="rd")
        maskt = const.tile([128, 128], BF16, tag="maskt", name="maskt")
        bselt = const.tile([65, 64], F32R, tag="bselt", name="bselt")
        xtT = persist.tile([128, NCK, T], BF16, tag="xtT", name="xtT")
        wqkT = persist.tile([128, NCK, 4 * 128], BF16, tag="wqkT", name="wqkT")
        wvT = persist.tile([128, NCK, HPC * D], BF16, tag="wvT", name="wvT")
        dummy = const.tile([128, 16], BF16, tag="dummy", name="dummy")
        warm = const.tile([128, TQ], BF16, tag="warm", name="warm")

        # ---- act-table preload (one-time ~2.7us) during the DMA window --
        nc.vector.memset(dummy[:], 0.0)
        nc.vector.memset(warm[:], 0.0)
        nc.scalar.activation(dummy[:], dummy[:], EXP, scale=1.0)

        # ---- input DMAs --------------------------------------------------
        # The first exp needs: q pair0 block0 (all xt b0 + wqk mc0 cols)
        # and cktT[0][:, 0:128].  Per-queue DMA bandwidth is ~22 GB/s, so
        # the 1MB xt block is split across SP (kc 0-3) and Pool (kc 4-7)
        # queues while ACT carries the small wqk-mc0 and ckt slivers.
        for kc_ in range(6):
            nc.sync.dma_start(xtT[:, kc_, 0:TQ], xt[:, kc_, 0:TQ])
        for kc_ in range(6, NCK):
            nc.gpsimd.dma_start(xtT[:, kc_, 0:TQ], xt[:, kc_, 0:TQ])
        nc.scalar.dma_start(cktT[0][:, 0:KC], ckt[0, :, 0:KC])
        for kc2 in range(4):
            nc.scalar.dma_start(wqkT[:, 2 * kc2:2 * kc2 + 2, 0:128],
                                wqk[:, 2 * kc2:2 * kc2 + 2, 0:128])
        # rest of pair0 prefix keys + values on ACT
        nc.scalar.dma_start(cktT[0][:, KC:2 * TQ], ckt[0, :, KC:2 * TQ])
        nc.scalar.dma_start(cktT[0][:, 2 * TQ:P], ckt[0, :, 2 * TQ:P])
        for j in range(4):
            nc.scalar.dma_start(cvt[0][:, :, 4 * j:4 * j + 4, :],
                                cv[0, :, :, 4 * j:4 * j + 4, :])
        # SP HWDGE (parallel queues): everything else, deadline order.
        for kc_ in range(NCK):
            nc.sync.dma_start(wqkT[:, kc_, 128:512], wqk[:, kc_, 128:512])
        for kc_ in range(NCK):
            nc.sync.dma_start(wvT[:, kc_, :], wv[:, kc_, :])
        for j in range(4):
            nc.sync.dma_start(cktT[1][:, j * TQ:(j + 1) * TQ],
                              ckt[1, :, j * TQ:(j + 1) * TQ])
        for j in range(4):
            nc.sync.dma_start(cvt[1][:, :, 4 * j:4 * j + 4, :],
                              cv[1, :, :, 4 * j:4 * j + 4, :])
        for kc_ in range(NCK):
            nc.sync.dma_start(xtT[:, kc_, TQ:2 * TQ], xt[:, kc_, TQ:2 * TQ])
        for pr in range(NPAIR):
            nc.sync.dma_start(wpt[pr][:], wp[pr, :, :])
        for nb in range(2, NT):
            for kc_ in range(NCK):
                nc.sync.dma_start(xtT[:, kc_, nb * TQ:(nb + 1) * TQ],
                                  xt[:, kc_, nb * TQ:(nb + 1) * TQ])

        # Pool SWDGE is ONE serial queue - only tiny/no-dep work here.
        for tc_ in range(NTC):
            nc.gpsimd.memset(vt[:, tc_, :, 64:66], 1.0)
        nc.gpsimd.dma_start(maskt[:], masks[:, :])

        with ExitStack() as main:
            psA = main.enter_context(tc.tile_pool(name="psA", bufs=2, space="PSUM"))
            psY = main.enter_context(tc.tile_pool(name="psY", bufs=1, space="PSUM"))
            ps1 = main.enter_context(tc.tile_pool(name="ps1", bufs=2, space="PSUM"))
            ebp = main.enter_context(tc.tile_pool(name="ebp", bufs=4))
            ybf = main.enter_context(tc.tile_pool(name="ybf", bufs=2))
            rbp = main.enter_context(tc.tile_pool(name="rbp", bufs=2))
            ytmp = main.enter_context(tc.tile_pool(name="ytmp", bufs=2))
            stg = main.enter_context(tc.tile_pool(name="stg", bufs=2))

            # ---- PE warmup: ~16 independent matmuls on garbage SBUF so
            # HAM un-throttles during the initial DMA wait.
            wps = psA.tile([128, 2, TQ], F32, tag="sb", name="wps")
            for i in range(8):
                nc.tensor.matmul(wps[:, i % 2, :], warm[:, 0:128], warm[:],
                                 start=True, stop=True, skip_group_check=True)

            # ---- phase-1 block thunks (cost, fn) lists -------------------
            def qk_block_thunks(mc, nb):
                dest = (qT[0], qT[1], kT[0], kT[1])[mc]
                cell = {}
                thunks = []

                def mk_mm(kc_):
                    def f():
                        if "ps" not in cell:
                            cell["ps"] = ps1.tile([128, TQ], F32, tag="p1", name="p1")
                        nc.tensor.matmul(
                            cell["ps"][:],
                            wqkT[:, kc_, mc * 128:(mc + 1) * 128],
                            xtT[:, kc_, nb * TQ:(nb + 1) * TQ],
                            start=(kc_ == 0), stop=(kc_ == NCK - 1),
                            skip_group_check=True)
                    return f

                for kc_ in range(NCK):
                    thunks.append((1.0, mk_mm(kc_)))

                def fin():
                    with nc.allow_low_precision(reason="q/k psum -> bf16 SBUF"):
                        nc.vector.tensor_copy(
                            dest[:, nb * TQ:(nb + 1) * TQ], cell["ps"][:])
                thunks.append((0.3, fin))
                return thunks

            def v_block_thunks(tc_):
                cell = {}
                thunks = []

                def mk_mm(kc_):
                    def f():
                        if "ps" not in cell:
                            cell["ps"] = ps1.tile([128, TQ], F32, tag="p1", name="p1")
                        nc.tensor.matmul(
                            cell["ps"][:, 0:HPC * D],
                            xtT[:, kc_, tc_ * 128:(tc_ + 1) * 128],
                            wvT[:, kc_, :],
                            start=(kc_ == 0), stop=(kc_ == NCK - 1),
                            skip_group_check=True)
                    return f

                for kc_ in range(NCK):
                    thunks.append((0.6, mk_mm(kc_)))

                def fin():
                    with nc.allow_low_precision(reason="v psum -> bf16 SBUF"):
                        nc.vector.tensor_copy(
                            vt[:, tc_, :, 0:64], cell["ps"][:, 0:HPC * D])
                thunks.append((0.3, fin))
                return thunks

            def proj_block_thunks(nb, mc):
                cell = {}
                thunks = []

                def mk_mm(pr):
                    def f():
                        if "ps" not in cell:
                            cell["ps"] = ps1.tile([128, TQ], F32, tag="p1", name="p1")
                        nc.tensor.matmul(
                            cell["ps"][:],
                            wpt[pr][:, mc * 128:(mc + 1) * 128],
                            ysb[pr][:, nb, :],
                            start=(pr == 0), stop=(pr == NPAIR - 1),
                            skip_group_check=True)
                    return f

                thunks.append((1.0, mk_mm(0)))
                thunks.append((1.0, mk_mm(1)))

                def fin():
                    ot = stg.tile([128, TQ], BF16, tag="ot", name="ot")
                    cell["ot"] = ot
                    with nc.allow_low_precision(reason="out psum -> bf16"):
                        nc.vector.tensor_copy(ot[:], cell["ps"][:])

                def dma():
                    qeng = nc.sync if mc % 2 == 0 else nc.scalar
                    qeng.dma_start(
                        out_t[mc * 128:(mc + 1) * 128, nb * TQ:(nb + 1) * TQ],
                        cell["ot"][:])
                thunks.append((0.3, fin))
                thunks.append((0.2, dma))
                return thunks

            # ---- chunk stream metadata -----------------------------------
            chunks = []
            seg_start = {}
            for tb in range(NT):
                for pr in range(NPAIR):
                    nkc = NPC + 4 * (tb + 1)
                    seg_start[(tb, pr)] = len(chunks)
                    for c in range(nkc):
                        chunks.append((tb, pr, c, nkc))
            NCH = len(chunks)  # 208
            boundary = set(seg_start.values()) - {0}

            # ---- extras FIFO (sorted by deadline, FIFO-emitted) ----------
            BIG = 10 ** 9
            items = []  # (sort_key, hard_deadline, min_slot, thunks)
            for nb in range(NT):
                if nb > 0:
                    d = seg_start[(nb, 0)] - 2
                    items.append((d, d, 0, qk_block_thunks(0, nb)))
                d = seg_start[(nb, 0)] + NPC - 2
                items.append((d, d, 0, qk_block_thunks(2, nb)))
                d = seg_start[(nb, 1)] - 2
                items.append((d, d, 0, qk_block_thunks(1, nb)))
                d = seg_start[(nb, 1)] + NPC - 2
                items.append((d, d, 0, qk_block_thunks(3, nb)))
            for tc_ in range(NTC):
                d = seg_start[(tc_ // 4, 0)] + NPC + tc_ + 1
                items.append((d, d, 0, v_block_thunks(tc_)))
            for nb in range(NT - 1):
                ms = seg_start[(nb + 1, 0)] + 14
                for mc in range(C // 128):
                    items.append((ms + 45, BIG, ms, proj_block_thunks(nb, mc)))
            items.sort(key=lambda it: it[0])
            eq = deque([dl, ms, deque(ths), False] for _, dl, ms, ths in items)

            def run_extras(slot, budget):
                while eq:
                    item = eq[0]
                    dl, ms, ths, _ = item
                    if not ths:
                        eq.popleft()
                        continue
                    if ms > slot or budget <= 0:
                        break
                    cost, fn = ths.popleft()
                    assert slot <= dl, f"extras deadline missed: {slot} > {dl}"
                    item[3] = True
                    fn()
                    budget -= cost

            # ---- upfront: q pair0 block 0 --------------------------------
            for _, th in qk_block_thunks(0, 0):
                th()

            # ---- per-chunk emitters --------------------------------------
            sb_ring = {}
            eb_ring = {}
            seg_state = {}

            def emit_scores(g):
                tb, pr, c, nkc = chunks[g]
                j = (c - NPC) - 4 * tb if c >= NPC else -1
                qlo = j * 128 if j >= 1 else 0
                sb = psA.tile([128, 2, TQ], F32, tag="sb", name="sb")
                sb_ring[g] = (sb, qlo, j)
                for h in range(2):
                    if c < NPC:
                        ksrc, klo = cktT[pr], c * KC
                    else:
                        ksrc, klo = kT[pr], (c - NPC) * KC
                    nc.tensor.matmul(
                        sb[:, h, qlo:TQ],
                        ksrc[h * 64:(h + 1) * 64, klo:klo + KC],
                        qT[pr][h * 64:(h + 1) * 64, tb * TQ + qlo:(tb + 1) * TQ],
                        start=True, stop=True,
                        tile_position=(h * 64, 0),
                        skip_group_check=True)

            def emit_exp(g):
                sb, qlo, j = sb_ring[g]
                eb = ebp.tile([128, 2, TQ], BF16, tag="eb", name="eb")
                eb_ring[g] = eb
                nc.scalar.activation(eb[:, :, qlo:TQ], sb[:, :, qlo:TQ],
                                     EXP, scale=SCALE)
                if j >= 0:
                    for h in range(2):
                        nc.vector.tensor_tensor(
                            eb[:, h, qlo:qlo + 128], eb[:, h, qlo:qlo + 128],
                            maskt[:, :], MULT)

            def emit_av(g, cur_slot):
                tb, pr, c, nkc = chunks[g]
                _, qlo, j = sb_ring.pop(g)
                eb = eb_ring.pop(g)
                if c == 0:
                    seg_state[(tb, pr)] = psY.tile([65, 2, TQ], F32, tag="y",
                                                   name="y")
                y = seg_state[(tb, pr)]
                st, sp = (c == 0), (c == nkc - 1)
                for hh in range(2):
                    if c < NPC:
                        vsrc = cvt[pr][:, hh, c, 0:65]
                    else:
                        vsrc = vt[:, c - NPC, 2 * pr + hh, 0:65]
                    nc.tensor.matmul(y[:, hh, qlo:TQ], vsrc, eb[:, hh, qlo:TQ],
                                     start=st, stop=sp,
                                     tile_position=(0, 0),
                                     skip_group_check=True)
                if sp:
                    emit_tail(tb, pr, y, cur_slot)

            def push_extra(item):
                # keep relative order; never cut into a partially emitted
                # ps1 block (ring corruption).
                pos = 1 if (eq and eq[0][3] and eq[0][2]) else 0
                eq.insert(pos, item)
                return pos

            def emit_tail(tb, pr, y, cur_slot):
                ybf_t = ybf.tile([65, 2, TQ], F32, tag="ybf", name="ybf")
                nc.vector.tensor_copy(ybf_t[:], y[:])

                # reciprocals: 4 half-row pieces (~1.7us DVE each) spread
                # as extras so the boundary DVE burst doesn't delay fins.
                def mk_recip(hh, lo, hi):
                    def f():
                        nc.vector.reciprocal(rd[64:65, hh, lo:hi],
                                             ybf_t[64:65, hh, lo:hi])
                    return f
                rth = deque((0.5, mk_recip(hh, lo, lo + TQ // 2))
                            for hh in (0, 1) for lo in (0, TQ // 2))

                def tail_b():
                    # broadcast r rows to 64 partitions via stride-0-source
                    # DMAs (no PE, no DVE) then normalize on DVE.
                    rbE = rbp.tile([64, TQ], F32, tag="rb", name="rbE")
                    rbO = rbp.tile([64, TQ], F32, tag="rb", name="rbO")
                    nc.sync.dma_start(
                        rbE[:],
                        rd[64:65, 0, :].unsqueeze(1).broadcast_to((1, 64, TQ)))
                    nc.scalar.dma_start(
                        rbO[:],
                        rd[64:65, 1, :].unsqueeze(1).broadcast_to((1, 64, TQ)))
                    with nc.allow_low_precision(reason="normalize -> bf16 y"):
                        nc.vector.tensor_tensor(
                            ysb[pr][0:64, tb, :], ybf_t[0:64, 0, :],
                            rbE[:], MULT)
                        yo = ytmp.tile([64, TQ], BF16, tag="yo", name="yo")
                        nc.vector.tensor_tensor(
                            yo[:], ybf_t[0:64, 1, :], rbO[:], MULT)
                    nc.sync.dma_start(ysb[pr][64:128, tb, :], yo[:])
                # tail_b only enters the PE queue once the recips are
                # nearly done (else the bcast MMs stall the PE ~6us and
                # HAM re-throttles).  Emission order recips -> tail_b is
                # mandatory (DVE FIFO: normalize must queue after recips).
                pos = push_extra([BIG, cur_slot + 1, rth, False])
                eq.insert(pos + 1, [BIG, cur_slot + 10,
                                    deque([(0.8, tail_b)]), False])

            # ---- the stream ----------------------------------------------
            for g in range(NCH + 2):
                tb, pr, c, nkc = chunks[min(g, NCH - 1)]
                diag = g < NCH and c >= NPC and (c - NPC) - 4 * tb >= 1
                if g < NCH:
                    emit_scores(g)
                    emit_exp(g)
                if g - 2 >= 0:
                    emit_av(g - 2, g)
                budget = 3.0 if (g < 20 or diag) else 2.0
                if g - c in boundary and c <= 2:
                    budget = 3.5   # keep the PE fed across the boundary
                run_extras(g, budget)

            # ---- drain remaining extras + final projection ---------------
            while eq:
                run_extras(NCH + 10**6, 100.0)
            # nb=3 projection: 4 waves of 2 mc through the (now free) psA
            # banks, one batched cast per wave, out DMAs on two queues.
            for w in range(4):
                fps = psA.tile([128, 2, TQ], F32, tag="sb", name="fps")
                for sub in range(2):
                    mc = 2 * w + sub
                    for pr in range(NPAIR):
                        nc.tensor.matmul(
                            fps[:, sub, :],
                            wpt[pr][:, mc * 128:(mc + 1) * 128],
                            ysb[pr][:, 3, :],
                            start=(pr == 0), stop=(pr == NPAIR - 1),
                            skip_group_check=True)
                fot = ebp.tile([128, 2, TQ], BF16, tag="eb", name="fot")
                with nc.allow_low_precision(reason="out psum -> bf16"):
                    nc.vector.tensor_copy(fot[:], fps[:])
                for sub in range(2):
                    mc = 2 * w + sub
                    qeng = nc.sync if sub == 0 else nc.scalar
                    qeng.dma_start(
                        out_t[mc * 128:(mc + 1) * 128, 3 * TQ:4 * TQ],
                        fot[:, sub, :])

    nc.compile()
    return nc


def make_in_maps(x, W_attn, W_proj, cache_k, cache_v, n_cores=8):
    """Shard full inputs into per-core input maps (host side)."""
    b_, t_, c_ = x.shape
    h_ = cache_k.shape[1]
    d_ = c_ // h_
    p_ = cache_k.shape[2]
    hpc = h_ // (n_cores // b_)
    Wq = W_attn[:, 0 * c_:1 * c_]
    Wk = W_attn[:, 1 * c_:2 * c_]
    Wv = W_attn[:, 2 * c_:3 * c_]
    # M0[p, q] = 1 if q >= p else 0 (shared by all diagonal chunks)
    mask_np = (np.arange(128)[None, :] >=
               np.arange(128)[:, None]).astype(np.float32)
    bsel_np = np.zeros((65, 64), np.float32)
    bsel_np[64, :] = 1.0
    in_maps = []
    for core in range(n_cores):
        b = core // (n_cores // b_)
        h0 = (core % (n_cores // b_)) * hpc
        heads = list(range(h0, h0 + hpc))
        cols = np.concatenate([np.arange(h * d_, (h + 1) * d_) for h in heads])
        # x^T chunked: xt[p, kc, t] = x[b, t, kc*128+p]
        xt_np = np.ascontiguousarray(
            x[b].T.reshape(NCK, 128, t_).transpose(1, 0, 2))
        # W cols: [q pair0 | q pair1 | k pair0 | k pair1], each 128 wide
        wqk_cols = np.concatenate(
            [Wq[:, cols[0:128]], Wq[:, cols[128:256]],
             Wk[:, cols[0:128]], Wk[:, cols[128:256]]], axis=1)
        wqk_np = np.ascontiguousarray(
            wqk_cols.reshape(NCK, 128, 512).transpose(1, 0, 2))
        wv_np = np.ascontiguousarray(
            Wv[:, cols].reshape(NCK, 128, 256).transpose(1, 0, 2))
        npair = hpc // 2
        ckt_np = np.zeros((npair, 128, p_), np.float32)
        cv_np = np.zeros((npair, 128, 2, NPC, 66), np.float32)
        wp_np = np.zeros((npair, 128, c_), np.float32)
        for pr in range(npair):
            he, ho = heads[2 * pr], heads[2 * pr + 1]
            ckt_np[pr, 0:64] = cache_k[b, he].T
            ckt_np[pr, 64:128] = cache_k[b, ho].T
            for hh, hd in ((0, he), (1, ho)):
                cvr = cache_v[b, hd].reshape(NPC, KC, d_)   # [chunk, key, d]
                cv_np[pr, :, hh, :, 0:64] = cvr.transpose(1, 0, 2)
                cv_np[pr, :, hh, :, 64] = 1.0
            wp_np[pr, 0:64] = W_proj[he * d_:(he + 1) * d_]
            wp_np[pr, 64:128] = W_proj[ho * d_:(ho + 1) * d_]
        in_maps.append({
            "xt": xt_np.astype(ml_dtypes.bfloat16),
            "wqk": wqk_np.astype(ml_dtypes.bfloat16),
            "wv": wv_np.astype(ml_dtypes.bfloat16),
            "ckt": ckt_np.astype(ml_dtypes.bfloat16),
            "cv": cv_np.astype(ml_dtypes.bfloat16),
            "wp": wp_np.astype(ml_dtypes.bfloat16),
            "masks": mask_np.astype(ml_dtypes.bfloat16),
            "bsel": bsel_np,
            "zrd": np.zeros((65, 2, TQ), np.float32),
        })
    return in_maps


def assemble_output(results, n_cores=8, b_=B, t_=T, c_=C):
    """Sum per-core partial out^T over head groups, transpose back."""
    out = np.zeros((b_, t_, c_), np.float32)
    per_b = n_cores // b_
    for b in range(b_):
        acc = np.zeros((c_, t_), np.float32)
        for i in range(per_b):
            acc += results[b * per_b + i]["out_t"].astype(np.float32)
        out[b] = acc.T
    return out


_NC_CACHE = {}


def kernel(x, W_attn, W_proj, cache_k, cache_v):
    x = np.asarray(x, np.float32)
    W_attn = np.asarray(W_attn, np.float32)
    W_proj = np.asarray(W_proj, np.float32)
    cache_k = np.asarray(cache_k, np.float32)
    cache_v = np.asarray(cache_v, np.float32)
    if "nc" not in _NC_CACHE:
        _NC_CACHE["nc"] = build_kernel()
    nc = _NC_CACHE["nc"]
    in_maps = make_in_maps(x, W_attn, W_proj, cache_k, cache_v)
    res = run_bass_kernel_spmd(nc, in_maps, list(range(8)))
    return assemble_output(res.results)


# revision 16
# speedup vs baseline: 1.0936x; 1.0936x over previous
"""Trainium2 Bass kernel for CausalSelfAttention with KV-prefix cache (v3).

Problem (hardcoded): B=2, T=2048, C=1024, H=16, D=64, P=2048.
Sharding: 8 cores = 2 (batch) x 4 (head groups of 4 heads).

v3 design vs v2 (354us measured):
 - ScalarE (exp) is the hard floor (~1 elem/cycle/lane @1.2GHz over 27M
   score elements ~= 178us busy).  v2 lost ~34us before the first exp,
   ~54us to ACT gaps at pair boundaries, and ~48us of tail after the
   last exp.  v3 restructures the whole kernel as ONE flat chunk stream
   with a uniform software pipeline: at slot g we emit scores(g), exp(g)
   and AV(g-2).  The 2-slot AV lag crosses segment (tb,pr) boundaries,
   so the PE always has independent work and ACT never waits.
 - causal trim: diagonal chunks only compute/exp/mask the query range
   [j*128, 512) (saves ~12% of exp and score/AV streaming).
 - QKV projection blocks, output-projection blocks and out DMAs are
   woven into the stream with a unit-cost budget + deadlines; only the
   nb=3 projection remains after the stream.
 - tail per segment: one [65,2,512] PSUM->SBUF copy, 2x
   reciprocal_approx_fast (DVE custom op, ~5x faster than reciprocal),
   2x gpsimd partition_broadcast (Pool, idle engine) and 2 DVE
   normalize mults.  No PE broadcast matmuls, no 3.3us reciprocals.
 - ScalarE act-table preload + 16 PE warmup matmuls during the initial
   DMA window (HAM warm, no 2.7us table load on the first real exp).
 - DMA issue spread across SP / ACT / Pool queues so the critical
   first tiles (wqk, xt block0, ckt pair0) land ASAP.
"""

import numpy as np
import ml_dtypes
from collections import deque
from contextlib import ExitStack

import concourse.bacc as bacc
import concourse.tile as tile
import concourse.mybir as mybir
from concourse.bass_utils import run_bass_kernel_spmd

F32 = mybir.dt.float32
F32R = mybir.dt.float32r
BF16 = mybir.dt.bfloat16
EXP = mybir.ActivationFunctionType.Exp
MULT = mybir.AluOpType.mult

B, T, C, H, D, P = 2, 2048, 1024, 16, 64, 2048
HPC = 4            # heads per core
NPAIR = 2          # head pairs per core
TQ = 512           # query block (matmul moving dim)
KC = 128           # key chunk (PSUM partition dim)
SCALE = 1.0 / np.sqrt(D)

NT = T // TQ       # 4  query blocks
NPC = P // KC      # 16 prefix key chunks
NCK = C // 128     # 8  C contraction chunks
NTC = T // 128     # 16 current-key 128-chunks

USE_POOL_BCAST = False  # extended-ISA ops are not supported by this runtime
                        # (verified: CoreSim passes, HW returns garbage)


def build_kernel(n_cores=8):
    nc = bacc.Bacc("TRN2", target_bir_lowering=False, debug=False,
                   num_devices=n_cores)

    xt = nc.dram_tensor("xt", [128, NCK, T], BF16, kind="ExternalInput").ap()
    wqk = nc.dram_tensor("wqk", [128, NCK, 4 * 128], BF16, kind="ExternalInput").ap()
    wv = nc.dram_tensor("wv", [128, NCK, HPC * D], BF16, kind="ExternalInput").ap()
    ckt = nc.dram_tensor("ckt", [NPAIR, 128, P], BF16, kind="ExternalInput").ap()
    cv = nc.dram_tensor("cv", [NPAIR, 128, 2, NPC, 66], BF16, kind="ExternalInput").ap()
    wp = nc.dram_tensor("wp", [NPAIR, 128, C], BF16, kind="ExternalInput").ap()
    masks = nc.dram_tensor("masks", [128, 128], BF16, kind="ExternalInput").ap()
    bsel = nc.dram_tensor("bsel", [65, 64], F32, kind="ExternalInput").ap()
    zrd = nc.dram_tensor("zrd", [65, 2, TQ], F32, kind="ExternalInput").ap()
    out_t = nc.dram_tensor("out_t", [C, T], BF16, kind="ExternalOutput").ap()

    with tile.TileContext(nc) as tc, ExitStack() as top:
        const = top.enter_context(tc.tile_pool(name="const", bufs=1))
        persist = top.enter_context(tc.tile_pool(name="persist", bufs=1))

        # ---- persistent SBUF ---------------------------------------------
        qT = [persist.tile([128, T], BF16, tag=f"qT{i}", name=f"qT{i}") for i in range(NPAIR)]
        kT = [persist.tile([128, T], BF16, tag=f"kT{i}", name=f"kT{i}") for i in range(NPAIR)]
        cktT = [persist.tile([128, P], BF16, tag=f"cktT{i}", name=f"cktT{i}") for i in range(NPAIR)]
        cvt = [persist.tile([128, 2, NPC, 66], BF16, tag=f"cvt{i}", name=f"cvt{i}") for i in range(NPAIR)]
        vt = persist.tile([128, NTC, HPC, 66], BF16, tag="vt", name="vt")
        wpt = [persist.tile([128, C], BF16, tag=f"wpt{i}", name=f"wpt{i}") for i in range(NPAIR)]
        ysb = [persist.tile([128, NT, TQ], BF16, tag=f"ysb{i}", name=f"ysb{i}") for i in range(NPAIR)]
        rd = persist.tile([65, 2, TQ], F32, tag="rd", name="rd")
        maskt = const.tile([128, 128], BF16, tag="maskt", name="maskt")
        bselt = const.tile([65, 64], F32R, tag="bselt", name="bselt")
        xtT = persist.tile([128, NCK, T], BF16, tag="xtT", name="xtT")
        wqkT = persist.tile([128, NCK, 4 * 128], BF16, tag="wqkT", name="wqkT")
        wvT = persist.tile([128, NCK, HPC * D], BF16, tag="wvT", name="wvT")
        dummy = const.tile([128, 16], BF16, tag="dummy", name="dummy")
        warm = const.tile([128, TQ], BF16, tag="warm", name="warm")

        # ---- act-table preload (one-time ~2.7us) during the DMA window --
        nc.vector.memset(dummy[:], 0.0)
        nc.vector.memset(warm[:], 0.0)
        nc.scalar.activation(dummy[:], dummy[:], EXP, scale=1.0)

        # ---- input DMAs --------------------------------------------------
        # The first exp needs: q pair0 block0 (all xt b0 + wqk mc0 cols)
        # and cktT[0][:, 0:128].  Per-queue DMA bandwidth is ~22 GB/s, so
        # the 1MB xt block is split across SP (kc 0-3) and Pool (kc 4-7)
        # queues while ACT carries the small wqk-mc0 and ckt slivers.
        for kc_ in range(6):
            nc.sync.dma_start(xtT[:, kc_, 0:TQ], xt[:, kc_, 0:TQ])
        for kc_ in range(6, NCK):
            nc.gpsimd.dma_start(xtT[:, kc_, 0:TQ], xt[:, kc_, 0:TQ])
        nc.scalar.dma_start(cktT[0][:, 0:KC], ckt[0, :, 0:KC])
        for kc2 in range(4):
            nc.scalar.dma_start(wqkT[:, 2 * kc2:2 * kc2 + 2, 0:128],
                                wqk[:, 2 * kc2:2 * kc2 + 2, 0:128])
        # rest of pair0 prefix keys + values on ACT
        nc.scalar.dma_start(cktT[0][:, KC:2 * TQ], ckt[0, :, KC:2 * TQ])
        nc.scalar.dma_start(cktT[0][:, 2 * TQ:P], ckt[0, :, 2 * TQ:P])
        for j in range(4):
            nc.scalar.dma_start(cvt[0][:, :, 4 * j:4 * j + 4, :],
                                cv[0, :, :, 4 * j:4 * j + 4, :])
        # SP HWDGE (parallel queues): everything else, deadline order.
        for kc_ in range(NCK):
            nc.sync.dma_start(wqkT[:, kc_, 128:512], wqk[:, kc_, 128:512])
        for kc_ in range(NCK):
            nc.sync.dma_start(wvT[:, kc_, :], wv[:, kc_, :])
        for j in range(4):
            nc.sync.dma_start(cktT[1][:, j * TQ:(j + 1) * TQ],
                              ckt[1, :, j * TQ:(j + 1) * TQ])
        for j in range(4):
            nc.sync.dma_start(cvt[1][:, :, 4 * j:4 * j + 4, :],
                              cv[1, :, :, 4 * j:4 * j + 4, :])
        for kc_ in range(NCK):
            nc.sync.dma_start(xtT[:, kc_, TQ:2 * TQ], xt[:, kc_, TQ:2 * TQ])
        for pr in range(NPAIR):
            nc.sync.dma_start(wpt[pr][:], wp[pr, :, :])
        for nb in range(2, NT):
            for kc_ in range(NCK):
                nc.sync.dma_start(xtT[:, kc_, nb * TQ:(nb + 1) * TQ],
                                  xt[:, kc_, nb * TQ:(nb + 1) * TQ])

        # Pool SWDGE is ONE serial queue - only tiny/no-dep work here.
        for tc_ in range(NTC):
            nc.gpsimd.memset(vt[:, tc_, :, 64:66], 1.0)
        nc.gpsimd.dma_start(maskt[:], masks[:, :])

        with ExitStack() as main:
            psA = main.enter_context(tc.tile_pool(name="psA", bufs=2, space="PSUM"))
            psY = main.enter_context(tc.tile_pool(name="psY", bufs=1, space="PSUM"))
            ps1 = main.enter_context(tc.tile_pool(name="ps1", bufs=2, space="PSUM"))
            ebp = main.enter_context(tc.tile_pool(name="ebp", bufs=4))
            ybf = main.enter_context(tc.tile_pool(name="ybf", bufs=2))
            rbp = main.enter_context(tc.tile_pool(name="rbp", bufs=2))
            ytmp = main.enter_context(tc.tile_pool(name="ytmp", bufs=2))
            stg = main.enter_context(tc.tile_pool(name="stg", bufs=2))

            # ---- PE warmup: ~16 independent matmuls on garbage SBUF so
            # HAM un-throttles during the initial DMA wait.
            wps = psA.tile([128, 2, TQ], F32, tag="sb", name="wps")
            for i in range(8):
                nc.tensor.matmul(wps[:, i % 2, :], warm[:, 0:128], warm[:],
                                 start=True, stop=True, skip_group_check=True)

            # ---- phase-1 block thunks (cost, fn) lists -------------------
            def qk_block_thunks(mc, nb):
                dest = (qT[0], qT[1], kT[0], kT[1])[mc]
                cell = {}
                thunks = []

                def mk_mm(kc_):
                    def f():
                        if "ps" not in cell:
                            cell["ps"] = ps1.tile([128, TQ], F32, tag="p1", name="p1")
                        nc.tensor.matmul(
                            cell["ps"][:],
                            wqkT[:, kc_, mc * 128:(mc + 1) * 128],
                            xtT[:, kc_, nb * TQ:(nb + 1) * TQ],
                            start=(kc_ == 0), stop=(kc_ == NCK - 1),
                            skip_group_check=True)
                    return f

                for kc_ in range(NCK):
                    thunks.append((1.0, mk_mm(kc_)))

                def fin():
                    with nc.allow_low_precision(reason="q/k psum -> bf16 SBUF"):
                        nc.vector.tensor_copy(
                            dest[:, nb * TQ:(nb + 1) * TQ], cell["ps"][:])
                thunks.append((0.3, fin))
                return thunks

            def v_block_thunks(tc_):
                cell = {}
                thunks = []

                def mk_mm(kc_):
                    def f():
                        if "ps" not in cell:
                            cell["ps"] = ps1.tile([128, TQ], F32, tag="p1", name="p1")
                        nc.tensor.matmul(
                            cell["ps"][:, 0:HPC * D],
                            xtT[:, kc_, tc_ * 128:(tc_ + 1) * 128],
                            wvT[:, kc_, :],
                            start=(kc_ == 0), stop=(kc_ == NCK - 1),
                            skip_group_check=True)
                    return f

                for kc_ in range(NCK):
                    thunks.append((0.6, mk_mm(kc_)))

                def fin():
                    with nc.allow_low_precision(reason="v psum -> bf16 SBUF"):
                        nc.vector.tensor_copy(
                            vt[:, tc_, :, 0:64], cell["ps"][:, 0:HPC * D])
                thunks.append((0.3, fin))
                return thunks

            def proj_block_thunks(nb, mc):
                cell = {}
                thunks = []

                def mk_mm(pr):
                    def f():
                        if "ps" not in cell:
                            cell["ps"] = ps1.tile([128, TQ], F32, tag="p1", name="p1")
                        nc.tensor.matmul(
                            cell["ps"][:],
                            wpt[pr][:, mc * 128:(mc + 1) * 128],
                            ysb[pr][:, nb, :],
                            start=(pr == 0), stop=(pr == NPAIR - 1),
                            skip_group_check=True)
                    return f

                thunks.append((1.0, mk_mm(0)))
                thunks.append((1.0, mk_mm(1)))

                def fin():
                    ot = stg.tile([128, TQ], BF16, tag="ot", name="ot")
                    cell["ot"] = ot
                    with nc.allow_low_precision(reason="out psum -> bf16"):
                        nc.vector.tensor_copy(ot[:], cell["ps"][:])

                def dma():
                    qeng = nc.sync
                    qeng.dma_start(
                        out_t[mc * 128:(mc + 1) * 128, nb * TQ:(nb + 1) * TQ],
                        cell["ot"][:])
                thunks.append((0.3, fin))
                thunks.append((0.2, dma))
                return thunks

            # ---- chunk stream metadata -----------------------------------
            chunks = []
            seg_start = {}
            for tb in range(NT):
                for pr in range(NPAIR):
                    nkc = NPC + 4 * (tb + 1)
                    seg_start[(tb, pr)] = len(chunks)
                    for c in range(nkc):
                        chunks.append((tb, pr, c, nkc))
            NCH = len(chunks)  # 208
            boundary = set(seg_start.values()) - {0}

            # ---- extras FIFO (sorted by deadline, FIFO-emitted) ----------
            BIG = 10 ** 9
            items = []  # (sort_key, hard_deadline, min_slot, thunks)
            for nb in range(NT):
                if nb > 0:
                    d = seg_start[(nb, 0)] - 2
                    items.append((d, d, 0, qk_block_thunks(0, nb)))
                d = seg_start[(nb, 0)] + NPC - 2
                items.append((d, d, 0, qk_block_thunks(2, nb)))
                d = seg_start[(nb, 1)] - 2
                items.append((d, d, 0, qk_block_thunks(1, nb)))
                d = seg_start[(nb, 1)] + NPC - 2
                items.append((d, d, 0, qk_block_thunks(3, nb)))
            for tc_ in range(NTC):
                d = seg_start[(tc_ // 4, 0)] + NPC + tc_ + 1
                items.append((d, d, 0, v_block_thunks(tc_)))
            for nb in range(NT - 1):
                ms = seg_start[(nb + 1, 0)] + 18
                for mc in range(C // 128):
                    items.append((ms + 45, BIG, ms, proj_block_thunks(nb, mc)))
            items.sort(key=lambda it: it[0])
            eq = deque([dl, ms, deque(ths), False] for _, dl, ms, ths in items)

            def run_extras(slot, budget):
                while eq:
                    item = eq[0]
                    dl, ms, ths, _ = item
                    if not ths:
                        eq.popleft()
                        continue
                    if ms > slot or budget <= 0:
                        break
                    cost, fn = ths.popleft()
                    assert slot <= dl, f"extras deadline missed: {slot} > {dl}"
                    item[3] = True
                    fn()
                    budget -= cost

            # ---- upfront: q pair0 block 0 --------------------------------
            for _, th in qk_block_thunks(0, 0):
                th()

            # ---- per-chunk emitters --------------------------------------
            sb_ring = {}
            eb_ring = {}
            seg_state = {}

            def emit_scores(g):
                tb, pr, c, nkc = chunks[g]
                j = (c - NPC) - 4 * tb if c >= NPC else -1
                qlo = j * 128 if j >= 1 else 0
                sb = psA.tile([128, 2, TQ], F32, tag="sb", name="sb")
                sb_ring[g] = (sb, qlo, j)
                for h in range(2):
                    if c < NPC:
                        ksrc, klo = cktT[pr], c * KC
                    else:
                        ksrc, klo = kT[pr], (c - NPC) * KC
                    nc.tensor.matmul(
                        sb[:, h, qlo:TQ],
                        ksrc[h * 64:(h + 1) * 64, klo:klo + KC],
                        qT[pr][h * 64:(h + 1) * 64, tb * TQ + qlo:(tb + 1) * TQ],
                        start=True, stop=True,
                        tile_position=(h * 64, 0),
                        skip_group_check=True)

            def emit_exp(g):
                sb, qlo, j = sb_ring[g]
                eb = ebp.tile([128, 2, TQ], BF16, tag="eb", name="eb")
                eb_ring[g] = eb
                nc.scalar.activation(eb[:, :, qlo:TQ], sb[:, :, qlo:TQ],
                                     EXP, scale=SCALE)
                if j >= 0:
                    for h in range(2):
                        nc.vector.tensor_tensor(
                            eb[:, h, qlo:qlo + 128], eb[:, h, qlo:qlo + 128],
                            maskt[:, :], MULT)

            def emit_av(g, cur_slot):
                tb, pr, c, nkc = chunks[g]
                _, qlo, j = sb_ring.pop(g)
                eb = eb_ring.pop(g)
                if c == 0:
                    seg_state[(tb, pr)] = psY.tile([65, 2, TQ], F32, tag="y",
                                                   name="y")
                y = seg_state[(tb, pr)]
                st, sp = (c == 0), (c == nkc - 1)
                for hh in range(2):
                    if c < NPC:
                        vsrc = cvt[pr][:, hh, c, 0:65]
                    else:
                        vsrc = vt[:, c - NPC, 2 * pr + hh, 0:65]
                    nc.tensor.matmul(y[:, hh, qlo:TQ], vsrc, eb[:, hh, qlo:TQ],
                                     start=st, stop=sp,
                                     tile_position=(0, 0),
                                     skip_group_check=True)
                if sp:
                    emit_tail(tb, pr, y, cur_slot)

            def push_extra(item):
                # keep relative order; never cut into a partially emitted
                # ps1 block (ring corruption).
                pos = 1 if (eq and eq[0][3] and eq[0][2]) else 0
                eq.insert(pos, item)
                return pos

            def emit_tail(tb, pr, y, cur_slot):
                ybf_t = ybf.tile([65, 2, TQ], F32, tag="ybf", name="ybf")
                nc.vector.tensor_copy(ybf_t[:], y[:])

                # reciprocals: 4 half-row pieces (~1.7us DVE each) spread
                # as extras so the boundary DVE burst doesn't delay fins.
                def mk_recip(hh, lo, hi):
                    def f():
                        nc.vector.reciprocal(rd[64:65, hh, lo:hi],
                                             ybf_t[64:65, hh, lo:hi])
                    return f
                rth = deque((0.5, mk_recip(hh, lo, lo + TQ // 2))
                            for hh in (0, 1) for lo in (0, TQ // 2))

                cell = {}

                def tail_dma():
                    # broadcast r rows to 64 partitions via stride-0-source
                    # DMAs (SP queue only - a dep-waiting DMA on the ACT
                    # queue head-blocks the exp stream).  Split in halves
                    # so two HW queues move each row (~3us not ~6us).
                    rbE = rbp.tile([64, TQ], F32, tag="rb", name="rbE")
                    rbO = rbp.tile([64, TQ], F32, tag="rb", name="rbO")
                    cell["rb"] = (rbE, rbO)
                    for hh, rb_t in ((0, rbE), (1, rbO)):
                        for lo in (0, TQ // 2):
                            nc.sync.dma_start(
                                rb_t[:, lo:lo + TQ // 2],
                                rd[64:65, hh, lo:lo + TQ // 2]
                                .unsqueeze(1).broadcast_to((1, 64, TQ // 2)))

                def tail_norm():
                    rbE, rbO = cell["rb"]
                    with nc.allow_low_precision(reason="normalize -> bf16 y"):
                        nc.vector.tensor_tensor(
                            ysb[pr][0:64, tb, :], ybf_t[0:64, 0, :],
                            rbE[:], MULT)
                        yo = ytmp.tile([64, TQ], BF16, tag="yo", name="yo")
                        nc.vector.tensor_tensor(
                            yo[:], ybf_t[0:64, 1, :], rbO[:], MULT)
                    nc.sync.dma_start(ysb[pr][64:128, tb, :], yo[:])
                # staged so no engine queue head-blocks: recips (DVE)
                # early, rb broadcast DMAs (SP) once recips are close,
                # normalize (DVE) once the DMAs are close.
                pos = push_extra([BIG, cur_slot + 1, rth, False])
                eq.insert(pos + 1, [BIG, cur_slot + 6,
                                    deque([(0.3, tail_dma)]), False])
                eq.insert(pos + 2, [BIG, cur_slot + 13,
                                    deque([(0.5, tail_norm)]), False])

            # ---- the stream ----------------------------------------------
            for g in range(NCH + 2):
                tb, pr, c, nkc = chunks[min(g, NCH - 1)]
                diag = g < NCH and c >= NPC and (c - NPC) - 4 * tb >= 1
                if g < NCH:
                    emit_scores(g)
                    emit_exp(g)
                if g - 2 >= 0:
                    emit_av(g - 2, g)
                budget = 3.0 if (g < 20 or diag) else 2.0
                if g - c in boundary and c <= 2:
                    budget = 3.5   # keep the PE fed across the boundary
                run_extras(g, budget)

            # ---- drain remaining extras + final projection ---------------
            while eq:
                run_extras(NCH + 10**6, 100.0)
            # nb=3 projection: 4 waves of 2 mc through the (now free) psA
            # banks, one batched cast per wave, out DMAs on two queues.
            for w in range(4):
                fps = psA.tile([128, 2, TQ], F32, tag="sb", name="fps")
                for sub in range(2):
                    mc = 2 * w + sub
                    for pr in range(NPAIR):
                        nc.tensor.matmul(
                            fps[:, sub, :],
                            wpt[pr][:, mc * 128:(mc + 1) * 128],
                            ysb[pr][:, 3, :],
                            start=(pr == 0), stop=(pr == NPAIR - 1),
                            skip_group_check=True)
                fot = ebp.tile([128, 2, TQ], BF16, tag="eb", name="fot")
                with nc.allow_low_precision(reason="out psum -> bf16"):
                    nc.vector.tensor_copy(fot[:], fps[:])
                for sub in range(2):
                    mc = 2 * w + sub
                    nc.sync.dma_start(
                        out_t[mc * 128:(mc + 1) * 128, 3 * TQ:4 * TQ],
                        fot[:, sub, :])

    nc.compile()
    return nc


def make_in_maps(x, W_attn, W_proj, cache_k, cache_v, n_cores=8):
    """Shard full inputs into per-core input maps (host side)."""
    b_, t_, c_ = x.shape
    h_ = cache_k.shape[1]
    d_ = c_ // h_
    p_ = cache_k.shape[2]
    hpc = h_ // (n_cores // b_)
    Wq = W_attn[:, 0 * c_:1 * c_]
    Wk = W_attn[:, 1 * c_:2 * c_]
    Wv = W_attn[:, 2 * c_:3 * c_]
    # M0[p, q] = 1 if q >= p else 0 (shared by all diagonal chunks)
    mask_np = (np.arange(128)[None, :] >=
               np.arange(128)[:, None]).astype(np.float32)
    bsel_np = np.zeros((65, 64), np.float32)
    bsel_np[64, :] = 1.0
    in_maps = []
    for core in range(n_cores):
        b = core // (n_cores // b_)
        h0 = (core % (n_cores // b_)) * hpc
        heads = list(range(h0, h0 + hpc))
        cols = np.concatenate([np.arange(h * d_, (h + 1) * d_) for h in heads])
        # x^T chunked: xt[p, kc, t] = x[b, t, kc*128+p]
        xt_np = np.ascontiguousarray(
            x[b].T.reshape(NCK, 128, t_).transpose(1, 0, 2))
        # W cols: [q pair0 | q pair1 | k pair0 | k pair1], each 128 wide
        wqk_cols = np.concatenate(
            [Wq[:, cols[0:128]], Wq[:, cols[128:256]],
             Wk[:, cols[0:128]], Wk[:, cols[128:256]]], axis=1)
        wqk_np = np.ascontiguousarray(
            wqk_cols.reshape(NCK, 128, 512).transpose(1, 0, 2))
        wv_np = np.ascontiguousarray(
            Wv[:, cols].reshape(NCK, 128, 256).transpose(1, 0, 2))
        npair = hpc // 2
        ckt_np = np.zeros((npair, 128, p_), np.float32)
        cv_np = np.zeros((npair, 128, 2, NPC, 66), np.float32)
        wp_np = np.zeros((npair, 128, c_), np.float32)
        for pr in range(npair):
            he, ho = heads[2 * pr], heads[2 * pr + 1]
            ckt_np[pr, 0:64] = cache_k[b, he].T
            ckt_np[pr, 64:128] = cache_k[b, ho].T
            for hh, hd in ((0, he), (1, ho)):
                cvr = cache_v[b, hd].reshape(NPC, KC, d_)   # [chunk, key, d]
                cv_np[pr, :, hh, :, 0:64] = cvr.transpose(1, 0, 2)
                cv_np[pr, :, hh, :, 64] = 1.0
            wp_np[pr, 0:64] = W_proj[he * d_:(he + 1) * d_]
            wp_np[pr, 64:128] = W_proj[ho * d_:(ho + 1) * d_]
        in_maps.append({
            "xt": xt_np.astype(ml_dtypes.bfloat16),
            "wqk": wqk_np.astype(ml_dtypes.bfloat16),
            "wv": wv_np.astype(ml_dtypes.bfloat16),
            "ckt": ckt_np.astype(ml_dtypes.bfloat16),
            "cv": cv_np.astype(ml_dtypes.bfloat16),
            "wp": wp_np.astype(ml_dtypes.bfloat16),
            "masks": mask_np.astype(ml_dtypes.bfloat16),
            "bsel": bsel_np,
            "zrd": np.zeros((65, 2, TQ), np.float32),
        })
    return in_maps


def assemble_output(results, n_cores=8, b_=B, t_=T, c_=C):
    """Sum per-core partial out^T over head groups, transpose back."""
    out = np.zeros((b_, t_, c_), np.float32)
    per_b = n_cores // b_
    for b in range(b_):
        acc = np.zeros((c_, t_), np.float32)
        for i in range(per_b):
            acc += results[b * per_b + i]["out_t"].astype(np.float32)
        out[b] = acc.T
    return out


_NC_CACHE = {}


def kernel(x, W_attn, W_proj, cache_k, cache_v):
    x = np.asarray(x, np.float32)
    W_attn = np.asarray(W_attn, np.float32)
    W_proj = np.asarray(W_proj, np.float32)
    cache_k = np.asarray(cache_k, np.float32)
    cache_v = np.asarray(cache_v, np.float32)
    if "nc" not in _NC_CACHE:
        _NC_CACHE["nc"] = build_kernel()
    nc = _NC_CACHE["nc"]
    in_maps = make_in_maps(x, W_attn, W_proj, cache_k, cache_v)
    res = run_bass_kernel_spmd(nc, in_maps, list(range(8)))
    return assemble_output(res.results)


# revision 17
# speedup vs baseline: 1.0945x; 1.0009x over previous
"""Trainium2 Bass kernel for CausalSelfAttention with KV-prefix cache (v3).

Problem (hardcoded): B=2, T=2048, C=1024, H=16, D=64, P=2048.
Sharding: 8 cores = 2 (batch) x 4 (head groups of 4 heads).

v3 design vs v2 (354us measured):
 - ScalarE (exp) is the hard floor (~1 elem/cycle/lane @1.2GHz over 27M
   score elements ~= 178us busy).  v2 lost ~34us before the first exp,
   ~54us to ACT gaps at pair boundaries, and ~48us of tail after the
   last exp.  v3 restructures the whole kernel as ONE flat chunk stream
   with a uniform software pipeline: at slot g we emit scores(g), exp(g)
   and AV(g-2).  The 2-slot AV lag crosses segment (tb,pr) boundaries,
   so the PE always has independent work and ACT never waits.
 - causal trim: diagonal chunks only compute/exp/mask the query range
   [j*128, 512) (saves ~12% of exp and score/AV streaming).
 - QKV projection blocks, output-projection blocks and out DMAs are
   woven into the stream with a unit-cost budget + deadlines; only the
   nb=3 projection remains after the stream.
 - tail per segment: one [65,2,512] PSUM->SBUF copy, 2x
   reciprocal_approx_fast (DVE custom op, ~5x faster than reciprocal),
   2x gpsimd partition_broadcast (Pool, idle engine) and 2 DVE
   normalize mults.  No PE broadcast matmuls, no 3.3us reciprocals.
 - ScalarE act-table preload + 16 PE warmup matmuls during the initial
   DMA window (HAM warm, no 2.7us table load on the first real exp).
 - DMA issue spread across SP / ACT / Pool queues so the critical
   first tiles (wqk, xt block0, ckt pair0) land ASAP.
"""

import numpy as np
import ml_dtypes
from collections import deque
from contextlib import ExitStack

import concourse.bacc as bacc
import concourse.tile as tile
import concourse.mybir as mybir
from concourse.bass_utils import run_bass_kernel_spmd

F32 = mybir.dt.float32
F32R = mybir.dt.float32r
BF16 = mybir.dt.bfloat16
EXP = mybir.ActivationFunctionType.Exp
MULT = mybir.AluOpType.mult

B, T, C, H, D, P = 2, 2048, 1024, 16, 64, 2048
HPC = 4            # heads per core
NPAIR = 2          # head pairs per core
TQ = 512           # query block (matmul moving dim)
KC = 128           # key chunk (PSUM partition dim)
SCALE = 1.0 / np.sqrt(D)

NT = T // TQ       # 4  query blocks
NPC = P // KC      # 16 prefix key chunks
NCK = C // 128     # 8  C contraction chunks
NTC = T // 128     # 16 current-key 128-chunks

USE_POOL_BCAST = False  # extended-ISA ops are not supported by this runtime
                        # (verified: CoreSim passes, HW returns garbage)


def build_kernel(n_cores=8):
    nc = bacc.Bacc("TRN2", target_bir_lowering=False, debug=False,
                   num_devices=n_cores)

    xt = nc.dram_tensor("xt", [128, NCK, T], BF16, kind="ExternalInput").ap()
    wqk = nc.dram_tensor("wqk", [128, NCK, 4 * 128], BF16, kind="ExternalInput").ap()
    wv = nc.dram_tensor("wv", [128, NCK, HPC * D], BF16, kind="ExternalInput").ap()
    ckt = nc.dram_tensor("ckt", [NPAIR, 128, P], BF16, kind="ExternalInput").ap()
    cv = nc.dram_tensor("cv", [NPAIR, 128, 2, NPC, 66], BF16, kind="ExternalInput").ap()
    wp = nc.dram_tensor("wp", [NPAIR, 128, C], BF16, kind="ExternalInput").ap()
    masks = nc.dram_tensor("masks", [128, 128], BF16, kind="ExternalInput").ap()
    bsel = nc.dram_tensor("bsel", [65, 64], F32, kind="ExternalInput").ap()
    zrd = nc.dram_tensor("zrd", [65, 2, TQ], F32, kind="ExternalInput").ap()
    out_t = nc.dram_tensor("out_t", [C, T], BF16, kind="ExternalOutput").ap()

    with tile.TileContext(nc) as tc, ExitStack() as top:
        const = top.enter_context(tc.tile_pool(name="const", bufs=1))
        persist = top.enter_context(tc.tile_pool(name="persist", bufs=1))

        # ---- persistent SBUF ---------------------------------------------
        qT = [persist.tile([128, T], BF16, tag=f"qT{i}", name=f"qT{i}") for i in range(NPAIR)]
        kT = [persist.tile([128, T], BF16, tag=f"kT{i}", name=f"kT{i}") for i in range(NPAIR)]
        cktT = [persist.tile([128, P], BF16, tag=f"cktT{i}", name=f"cktT{i}") for i in range(NPAIR)]
        cvt = [persist.tile([128, 2, NPC, 66], BF16, tag=f"cvt{i}", name=f"cvt{i}") for i in range(NPAIR)]
        vt = persist.tile([128, NTC, HPC, 66], BF16, tag="vt", name="vt")
        wpt = [persist.tile([128, C], BF16, tag=f"wpt{i}", name=f"wpt{i}") for i in range(NPAIR)]
        ysb = [persist.tile([128, NT, TQ], BF16, tag=f"ysb{i}", name=f"ysb{i}") for i in range(NPAIR)]
        rd = persist.tile([65, 2, TQ], F32, tag="rd", name="rd")
        maskt = const.tile([128, 128], BF16, tag="maskt", name="maskt")
        bselt = const.tile([65, 64], F32, tag="bselt", name="bselt")
        xtT = persist.tile([128, NCK, T], BF16, tag="xtT", name="xtT")
        wqkT = persist.tile([128, NCK, 4 * 128], BF16, tag="wqkT", name="wqkT")
        wvT = persist.tile([128, NCK, HPC * D], BF16, tag="wvT", name="wvT")
        dummy = const.tile([128, 16], BF16, tag="dummy", name="dummy")
        warm = const.tile([128, TQ], BF16, tag="warm", name="warm")

        # ---- act-table preload (one-time ~2.7us) during the DMA window --
        nc.vector.memset(dummy[:], 0.0)
        nc.vector.memset(warm[:], 0.0)
        nc.scalar.activation(dummy[:], dummy[:], EXP, scale=1.0)

        # ---- input DMAs --------------------------------------------------
        # The first exp needs: q pair0 block0 (all xt b0 + wqk mc0 cols)
        # and cktT[0][:, 0:128].  Per-queue DMA bandwidth is ~22 GB/s, so
        # the 1MB xt block is split across SP (kc 0-3) and Pool (kc 4-7)
        # queues while ACT carries the small wqk-mc0 and ckt slivers.
        for kc_ in range(6):
            nc.sync.dma_start(xtT[:, kc_, 0:TQ], xt[:, kc_, 0:TQ])
        for kc_ in range(6, NCK):
            nc.gpsimd.dma_start(xtT[:, kc_, 0:TQ], xt[:, kc_, 0:TQ])
        nc.scalar.dma_start(cktT[0][:, 0:KC], ckt[0, :, 0:KC])
        for kc2 in range(4):
            nc.scalar.dma_start(wqkT[:, 2 * kc2:2 * kc2 + 2, 0:128],
                                wqk[:, 2 * kc2:2 * kc2 + 2, 0:128])
        # rest of pair0 prefix keys + values on ACT
        nc.scalar.dma_start(cktT[0][:, KC:2 * TQ], ckt[0, :, KC:2 * TQ])
        nc.scalar.dma_start(cktT[0][:, 2 * TQ:P], ckt[0, :, 2 * TQ:P])
        for j in range(4):
            nc.scalar.dma_start(cvt[0][:, :, 4 * j:4 * j + 4, :],
                                cv[0, :, :, 4 * j:4 * j + 4, :])
        # SP HWDGE (parallel queues): everything else, deadline order.
        for kc_ in range(NCK):
            nc.sync.dma_start(wqkT[:, kc_, 128:512], wqk[:, kc_, 128:512])
        for kc_ in range(NCK):
            nc.sync.dma_start(wvT[:, kc_, :], wv[:, kc_, :])
        for j in range(4):
            nc.sync.dma_start(cktT[1][:, j * TQ:(j + 1) * TQ],
                              ckt[1, :, j * TQ:(j + 1) * TQ])
        for j in range(4):
            nc.sync.dma_start(cvt[1][:, :, 4 * j:4 * j + 4, :],
                              cv[1, :, :, 4 * j:4 * j + 4, :])
        for kc_ in range(NCK):
            nc.sync.dma_start(xtT[:, kc_, TQ:2 * TQ], xt[:, kc_, TQ:2 * TQ])
        for pr in range(NPAIR):
            nc.sync.dma_start(wpt[pr][:], wp[pr, :, :])
        nc.sync.dma_start(bselt[:], bsel[:, :])
        nc.sync.dma_start(rd[:], zrd[:, :, :])
        for nb in range(2, NT):
            for kc_ in range(NCK):
                nc.sync.dma_start(xtT[:, kc_, nb * TQ:(nb + 1) * TQ],
                                  xt[:, kc_, nb * TQ:(nb + 1) * TQ])

        # Pool SWDGE is ONE serial queue - only tiny/no-dep work here.
        for tc_ in range(NTC):
            nc.gpsimd.memset(vt[:, tc_, :, 64:66], 1.0)
        nc.gpsimd.dma_start(maskt[:], masks[:, :])

        with ExitStack() as main:
            psA = main.enter_context(tc.tile_pool(name="psA", bufs=2, space="PSUM"))
            psY = main.enter_context(tc.tile_pool(name="psY", bufs=1, space="PSUM"))
            ps1 = main.enter_context(tc.tile_pool(name="ps1", bufs=2, space="PSUM"))
            ebp = main.enter_context(tc.tile_pool(name="ebp", bufs=4))
            ybf = main.enter_context(tc.tile_pool(name="ybf", bufs=2))
            rbp = main.enter_context(tc.tile_pool(name="rbp", bufs=2))
            ytmp = main.enter_context(tc.tile_pool(name="ytmp", bufs=2))
            stg = main.enter_context(tc.tile_pool(name="stg", bufs=2))

            # ---- PE warmup: ~16 independent matmuls on garbage SBUF so
            # HAM un-throttles during the initial DMA wait.
            wps = psA.tile([128, 2, TQ], F32, tag="sb", name="wps")
            for i in range(8):
                nc.tensor.matmul(wps[:, i % 2, :], warm[:, 0:128], warm[:],
                                 start=True, stop=True, skip_group_check=True)

            # ---- phase-1 block thunks (cost, fn) lists -------------------
            def qk_block_thunks(mc, nb):
                dest = (qT[0], qT[1], kT[0], kT[1])[mc]
                cell = {}
                thunks = []

                def mk_mm(kc_):
                    def f():
                        if "ps" not in cell:
                            cell["ps"] = ps1.tile([128, TQ], F32, tag="p1", name="p1")
                        nc.tensor.matmul(
                            cell["ps"][:],
                            wqkT[:, kc_, mc * 128:(mc + 1) * 128],
                            xtT[:, kc_, nb * TQ:(nb + 1) * TQ],
                            start=(kc_ == 0), stop=(kc_ == NCK - 1),
                            skip_group_check=True)
                    return f

                for kc_ in range(NCK):
                    thunks.append((1.0, mk_mm(kc_)))

                def fin():
                    with nc.allow_low_precision(reason="q/k psum -> bf16 SBUF"):
                        nc.vector.tensor_copy(
                            dest[:, nb * TQ:(nb + 1) * TQ], cell["ps"][:])
                thunks.append((0.3, fin))
                return thunks

            def v_block_thunks(tc_):
                cell = {}
                thunks = []

                def mk_mm(kc_):
                    def f():
                        if "ps" not in cell:
                            cell["ps"] = ps1.tile([128, TQ], F32, tag="p1", name="p1")
                        nc.tensor.matmul(
                            cell["ps"][:, 0:HPC * D],
                            xtT[:, kc_, tc_ * 128:(tc_ + 1) * 128],
                            wvT[:, kc_, :],
                            start=(kc_ == 0), stop=(kc_ == NCK - 1),
                            skip_group_check=True)
                    return f

                for kc_ in range(NCK):
                    thunks.append((0.6, mk_mm(kc_)))

                def fin():
                    with nc.allow_low_precision(reason="v psum -> bf16 SBUF"):
                        nc.vector.tensor_copy(
                            vt[:, tc_, :, 0:64], cell["ps"][:, 0:HPC * D])
                thunks.append((0.3, fin))
                return thunks

            def proj_block_thunks(nb, mc):
                cell = {}
                thunks = []

                def mk_mm(pr):
                    def f():
                        if "ps" not in cell:
                            cell["ps"] = ps1.tile([128, TQ], F32, tag="p1", name="p1")
                        nc.tensor.matmul(
                            cell["ps"][:],
                            wpt[pr][:, mc * 128:(mc + 1) * 128],
                            ysb[pr][:, nb, :],
                            start=(pr == 0), stop=(pr == NPAIR - 1),
                            skip_group_check=True)
                    return f

                thunks.append((1.0, mk_mm(0)))
                thunks.append((1.0, mk_mm(1)))

                def fin():
                    ot = stg.tile([128, TQ], BF16, tag="ot", name="ot")
                    cell["ot"] = ot
                    with nc.allow_low_precision(reason="out psum -> bf16"):
                        nc.vector.tensor_copy(ot[:], cell["ps"][:])

                def dma():
                    qeng = nc.sync
                    qeng.dma_start(
                        out_t[mc * 128:(mc + 1) * 128, nb * TQ:(nb + 1) * TQ],
                        cell["ot"][:])
                thunks.append((0.3, fin))
                thunks.append((0.2, dma))
                return thunks

            # ---- chunk stream metadata -----------------------------------
            chunks = []
            seg_start = {}
            for tb in range(NT):
                for pr in range(NPAIR):
                    nkc = NPC + 4 * (tb + 1)
                    seg_start[(tb, pr)] = len(chunks)
                    for c in range(nkc):
                        chunks.append((tb, pr, c, nkc))
            NCH = len(chunks)  # 208
            boundary = set(seg_start.values()) - {0}

            # ---- extras FIFO (sorted by deadline, FIFO-emitted) ----------
            BIG = 10 ** 9
            items = []  # (sort_key, hard_deadline, min_slot, thunks)
            for nb in range(NT):
                if nb > 0:
                    d = seg_start[(nb, 0)] - 2
                    items.append((d, d, 0, qk_block_thunks(0, nb)))
                d = seg_start[(nb, 0)] + NPC - 2
                items.append((d, d, 0, qk_block_thunks(2, nb)))
                d = seg_start[(nb, 1)] - 2
                items.append((d, d, 0, qk_block_thunks(1, nb)))
                d = seg_start[(nb, 1)] + NPC - 2
                items.append((d, d, 0, qk_block_thunks(3, nb)))
            for tc_ in range(NTC):
                d = seg_start[(tc_ // 4, 0)] + NPC + tc_ + 1
                items.append((d, d, 0, v_block_thunks(tc_)))
            for nb in range(NT - 1):
                ms = seg_start[(nb + 1, 0)] + 18
                for mc in range(C // 128):
                    items.append((ms + 45, BIG, ms, proj_block_thunks(nb, mc)))
            items.sort(key=lambda it: it[0])
            eq = deque([dl, ms, deque(ths), False] for _, dl, ms, ths in items)

            def run_extras(slot, budget):
                while eq:
                    item = eq[0]
                    dl, ms, ths, _ = item
                    if not ths:
                        eq.popleft()
                        continue
                    if ms > slot or budget <= 0:
                        break
                    cost, fn = ths.popleft()
                    assert slot <= dl, f"extras deadline missed: {slot} > {dl}"
                    item[3] = True
                    fn()
                    budget -= cost

            # ---- upfront: q pair0 block 0 --------------------------------
            for _, th in qk_block_thunks(0, 0):
                th()

            # ---- per-chunk emitters --------------------------------------
            sb_ring = {}
            eb_ring = {}
            seg_state = {}

            def emit_scores(g):
                tb, pr, c, nkc = chunks[g]
                j = (c - NPC) - 4 * tb if c >= NPC else -1
                qlo = j * 128 if j >= 1 else 0
                sb = psA.tile([128, 2, TQ], F32, tag="sb", name="sb")
                sb_ring[g] = (sb, qlo, j)
                for h in range(2):
                    if c < NPC:
                        ksrc, klo = cktT[pr], c * KC
                    else:
                        ksrc, klo = kT[pr], (c - NPC) * KC
                    nc.tensor.matmul(
                        sb[:, h, qlo:TQ],
                        ksrc[h * 64:(h + 1) * 64, klo:klo + KC],
                        qT[pr][h * 64:(h + 1) * 64, tb * TQ + qlo:(tb + 1) * TQ],
                        start=True, stop=True,
                        tile_position=(h * 64, 0),
                        skip_group_check=True)

            def emit_exp(g):
                sb, qlo, j = sb_ring[g]
                eb = ebp.tile([128, 2, TQ], BF16, tag="eb", name="eb")
                eb_ring[g] = eb
                nc.scalar.activation(eb[:, :, qlo:TQ], sb[:, :, qlo:TQ],
                                     EXP, scale=SCALE)
                if j >= 0:
                    for h in range(2):
                        nc.vector.tensor_tensor(
                            eb[:, h, qlo:qlo + 128], eb[:, h, qlo:qlo + 128],
                            maskt[:, :], MULT)

            def emit_av(g, cur_slot):
                tb, pr, c, nkc = chunks[g]
                _, qlo, j = sb_ring.pop(g)
                eb = eb_ring.pop(g)
                if c == 0:
                    seg_state[(tb, pr)] = psY.tile([65, 2, TQ], F32, tag="y",
                                                   name="y")
                y = seg_state[(tb, pr)]
                st, sp = (c == 0), (c == nkc - 1)
                for hh in range(2):
                    if c < NPC:
                        vsrc = cvt[pr][:, hh, c, 0:65]
                    else:
                        vsrc = vt[:, c - NPC, 2 * pr + hh, 0:65]
                    nc.tensor.matmul(y[:, hh, qlo:TQ], vsrc, eb[:, hh, qlo:TQ],
                                     start=st, stop=sp,
                                     tile_position=(0, 0),
                                     skip_group_check=True)
                if sp:
                    emit_tail(tb, pr, y, cur_slot)

            def push_extra(item):
                # keep relative order; never cut into a partially emitted
                # ps1 block (ring corruption).
                pos = 1 if (eq and eq[0][3] and eq[0][2]) else 0
                eq.insert(pos, item)
                return pos

            def emit_tail(tb, pr, y, cur_slot):
                ybf_t = ybf.tile([65, 2, TQ], F32, tag="ybf", name="ybf")
                nc.vector.tensor_copy(ybf_t[:], y[:])

                # reciprocals: 4 half-row pieces (~1.7us DVE each) spread
                # as extras so the boundary DVE burst doesn't delay fins.
                def mk_recip(hh, lo, hi):
                    def f():
                        nc.vector.reciprocal(rd[64:65, hh, lo:hi],
                                             ybf_t[64:65, hh, lo:hi])
                    return f
                rth = deque((1.0, mk_recip(hh, lo, lo + TQ // 2))
                            for hh in (0, 1) for lo in (0, TQ // 2))

                def tail_b():
                    # PE broadcast matmuls; scheduled late enough that the
                    # recips are done when the PE queue reaches them.
                    bcp = ps1.tile([128, TQ], F32, tag="p1", name="bcp")
                    bcp2 = ps1.tile([128, TQ], F32, tag="p1", name="bcp2")
                    nc.tensor.matmul(bcp[0:64, :], bselt[:], rd[:, 0, :],
                                     start=True, stop=True,
                                     skip_group_check=True)
                    with nc.allow_low_precision(reason="normalize -> bf16 y"):
                        nc.vector.tensor_tensor(
                            ysb[pr][0:64, tb, :], ybf_t[0:64, 0, :],
                            bcp[0:64, :], MULT)
                    nc.tensor.matmul(bcp2[0:64, :], bselt[:], rd[:, 1, :],
                                     start=True, stop=True,
                                     skip_group_check=True)
                    with nc.allow_low_precision(reason="normalize -> bf16 y"):
                        yo = ytmp.tile([64, TQ], BF16, tag="yo", name="yo")
                        nc.vector.tensor_tensor(
                            yo[:], ybf_t[0:64, 1, :], bcp2[0:64, :], MULT)
                    # yo via Pool SWDGE: its own sems, so this dep-waiting
                    # DMA can't head-block the SP input queue.
                    nc.gpsimd.dma_start(ysb[pr][64:128, tb, :], yo[:])
                # recips early (DVE), bcast+normalize once the recips
                # are certainly done (PE in-order queue must not stall).
                pos = push_extra([BIG, cur_slot + 1, rth, False])
                eq.insert(pos + 1, [BIG, cur_slot + 13,
                                    deque([(0.8, tail_b)]), False])

            # ---- the stream ----------------------------------------------
            for g in range(NCH + 2):
                tb, pr, c, nkc = chunks[min(g, NCH - 1)]
                diag = g < NCH and c >= NPC and (c - NPC) - 4 * tb >= 1
                if g < NCH:
                    emit_scores(g)
                    emit_exp(g)
                if g - 2 >= 0:
                    emit_av(g - 2, g)
                budget = 3.0 if (g < 20 or diag) else 2.0
                if g - c in boundary and c <= 2:
                    budget = 3.5   # keep the PE fed across the boundary
                run_extras(g, budget)

            # ---- drain remaining extras + final projection ---------------
            while eq:
                run_extras(NCH + 10**6, 100.0)
            # nb=3 projection: 4 waves of 2 mc through the (now free) psA
            # banks, one batched cast per wave, out DMAs on two queues.
            for w in range(4):
                fps = psA.tile([128, 2, TQ], F32, tag="sb", name="fps")
                for sub in range(2):
                    mc = 2 * w + sub
                    for pr in range(NPAIR):
                        nc.tensor.matmul(
                            fps[:, sub, :],
                            wpt[pr][:, mc * 128:(mc + 1) * 128],
                            ysb[pr][:, 3, :],
                            start=(pr == 0), stop=(pr == NPAIR - 1),
                            skip_group_check=True)
                fot = ebp.tile([128, 2, TQ], BF16, tag="eb", name="fot")
                with nc.allow_low_precision(reason="out psum -> bf16"):
                    nc.vector.tensor_copy(fot[:], fps[:])
                for sub in range(2):
                    mc = 2 * w + sub
                    nc.sync.dma_start(
                        out_t[mc * 128:(mc + 1) * 128, 3 * TQ:4 * TQ],
                        fot[:, sub, :])

    nc.compile()
    return nc


def make_in_maps(x, W_attn, W_proj, cache_k, cache_v, n_cores=8):
    """Shard full inputs into per-core input maps (host side)."""
    b_, t_, c_ = x.shape
    h_ = cache_k.shape[1]
    d_ = c_ // h_
    p_ = cache_k.shape[2]
    hpc = h_ // (n_cores // b_)
    Wq = W_attn[:, 0 * c_:1 * c_]
    Wk = W_attn[:, 1 * c_:2 * c_]
    Wv = W_attn[:, 2 * c_:3 * c_]
    # M0[p, q] = 1 if q >= p else 0 (shared by all diagonal chunks)
    mask_np = (np.arange(128)[None, :] >=
               np.arange(128)[:, None]).astype(np.float32)
    bsel_np = np.zeros((65, 64), np.float32)
    bsel_np[64, :] = 1.0
    in_maps = []
    for core in range(n_cores):
        b = core // (n_cores // b_)
        h0 = (core % (n_cores // b_)) * hpc
        heads = list(range(h0, h0 + hpc))
        cols = np.concatenate([np.arange(h * d_, (h + 1) * d_) for h in heads])
        # x^T chunked: xt[p, kc, t] = x[b, t, kc*128+p]
        xt_np = np.ascontiguousarray(
            x[b].T.reshape(NCK, 128, t_).transpose(1, 0, 2))
        # W cols: [q pair0 | q pair1 | k pair0 | k pair1], each 128 wide
        wqk_cols = np.concatenate(
            [Wq[:, cols[0:128]], Wq[:, cols[128:256]],
             Wk[:, cols[0:128]], Wk[:, cols[128:256]]], axis=1)
        wqk_np = np.ascontiguousarray(
            wqk_cols.reshape(NCK, 128, 512).transpose(1, 0, 2))
        wv_np = np.ascontiguousarray(
            Wv[:, cols].reshape(NCK, 128, 256).transpose(1, 0, 2))
        npair = hpc // 2
        ckt_np = np.zeros((npair, 128, p_), np.float32)
        cv_np = np.zeros((npair, 128, 2, NPC, 66), np.float32)
        wp_np = np.zeros((npair, 128, c_), np.float32)
        for pr in range(npair):
            he, ho = heads[2 * pr], heads[2 * pr + 1]
            ckt_np[pr, 0:64] = cache_k[b, he].T
            ckt_np[pr, 64:128] = cache_k[b, ho].T
            for hh, hd in ((0, he), (1, ho)):
                cvr = cache_v[b, hd].reshape(NPC, KC, d_)   # [chunk, key, d]
                cv_np[pr, :, hh, :, 0:64] = cvr.transpose(1, 0, 2)
                cv_np[pr, :, hh, :, 64] = 1.0
            wp_np[pr, 0:64] = W_proj[he * d_:(he + 1) * d_]
            wp_np[pr, 64:128] = W_proj[ho * d_:(ho + 1) * d_]
        in_maps.append({
            "xt": xt_np.astype(ml_dtypes.bfloat16),
            "wqk": wqk_np.astype(ml_dtypes.bfloat16),
            "wv": wv_np.astype(ml_dtypes.bfloat16),
            "ckt": ckt_np.astype(ml_dtypes.bfloat16),
            "cv": cv_np.astype(ml_dtypes.bfloat16),
            "wp": wp_np.astype(ml_dtypes.bfloat16),
            "masks": mask_np.astype(ml_dtypes.bfloat16),
            "bsel": bsel_np,
            "zrd": np.zeros((65, 2, TQ), np.float32),
        })
    return in_maps


def assemble_output(results, n_cores=8, b_=B, t_=T, c_=C):
    """Sum per-core partial out^T over head groups, transpose back."""
    out = np.zeros((b_, t_, c_), np.float32)
    per_b = n_cores // b_
    for b in range(b_):
        acc = np.zeros((c_, t_), np.float32)
        for i in range(per_b):
            acc += results[b * per_b + i]["out_t"].astype(np.float32)
        out[b] = acc.T
    return out


_NC_CACHE = {}


def kernel(x, W_attn, W_proj, cache_k, cache_v):
    x = np.asarray(x, np.float32)
    W_attn = np.asarray(W_attn, np.float32)
    W_proj = np.asarray(W_proj, np.float32)
    cache_k = np.asarray(cache_k, np.float32)
    cache_v = np.asarray(cache_v, np.float32)
    if "nc" not in _NC_CACHE:
        _NC_CACHE["nc"] = build_kernel()
    nc = _NC_CACHE["nc"]
    in_maps = make_in_maps(x, W_attn, W_proj, cache_k, cache_v)
    res = run_bass_kernel_spmd(nc, in_maps, list(range(8)))
    return assemble_output(res.results)
